# revision 5
# baseline (speedup 1.0000x reference)
"""Multi-head attention (B=4, S=2048, d_model=1024, H=16) on 8 TRN2 NeuronCores.

Sharding: tensor-parallel over heads x data-parallel over batch.
Core c handles batch b=c//2 and head group g=c%2 (8 heads = 512 of the
1024 d_model columns of W_Q/W_K/W_V, and 512 rows of W_O). Each core
produces a partial output Y_partial[b] = O_g @ W_O[g-rows, :] in fp16;
the host sums the two partials per batch in fp32.

Device-side dataflow per core (matmul operands fp16, accum fp32):
  - the attention phase is exp-bound (ScalarE ~1.1us per 128x1024 tile);
    projections and the output projection are interleaved into the PE's
    idle slots of that phase via a deadline-scheduled closure list
  - per head pair j, per 128-ktok block: scores^T = k^T.T q^T (row-tiled
    pair, concurrent in the PE array) -> exp(scale=1/8) -> P^T fp16
    -> out^T_ext += [v_h | 1].T @ P^T (row 64 = softmax denominator)
  - normalize: reciprocal_approx_fast on DVE, broadcast + multiply on
    the (otherwise idle) GpSimd engine
  - vt streams through a 6-deep rotating SBUF slice buffer (16 slices
    of 128 ktok); kt/qt/weights are SBUF-resident
"""

import numpy as np

B = 4
S = 2048
D = 1024
H = 16
DK = 64
NCORES = 8
HPC = 8          # heads per core
GCOLS = 512      # d_model columns per head group
QB = 512         # q-token block (PSUM bank free dim)
NQB = S // QB    # 4
NKB = S // 128   # 16 k-token blocks
NC_CHUNKS = D // 128  # 8 contraction chunks

_prog_cache = {}


def build_program(reps=1):
    """Build + compile the SPMD program."""
    key = (reps,)
    if key in _prog_cache:
        return _prog_cache[key]

    import concourse.bacc as bacc
    import concourse.mybir as mybir
    from concourse.tile import TileContext

    f16 = mybir.dt.float16
    f32 = mybir.dt.float32
    EXP = mybir.ActivationFunctionType.Exp

    nc = bacc.Bacc("TRN2", target_bir_lowering=False, debug=False,
                   num_devices=NCORES)

    qt_d = nc.dram_tensor("qt", [128, NC_CHUNKS, S], f16, kind="ExternalInput").ap()
    kt_d = nc.dram_tensor("kt", [128, NC_CHUNKS, S], f16, kind="ExternalInput").ap()
    vt_d = nc.dram_tensor("vt", [128, NC_CHUNKS, S], f16, kind="ExternalInput").ap()
    wq_d = nc.dram_tensor("wq", [128, NC_CHUNKS, GCOLS], f16, kind="ExternalInput").ap()
    wk_d = nc.dram_tensor("wk", [128, NC_CHUNKS, GCOLS], f16, kind="ExternalInput").ap()
    wv_d = nc.dram_tensor("wv", [128, NC_CHUNKS, GCOLS], f16, kind="ExternalInput").ap()
    wo_d = nc.dram_tensor("wo", [128, 4, D], f16, kind="ExternalInput").ap()
    yp_d = nc.dram_tensor("yp", [S, D], f16, kind="ExternalOutput").ap()

    with TileContext(nc) as tc:
        with tc.tile_pool(name="weights", bufs=1) as wpool, \
             tc.tile_pool(name="xin", bufs=1) as xpool, \
             tc.tile_pool(name="proj", bufs=1) as projpool, \
             tc.tile_pool(name="work", bufs=2) as workpool, \
             tc.tile_pool(name="psum", bufs=1, space="PSUM") as psp:

            wq_sb = wpool.tile([128, NC_CHUNKS, GCOLS], f16, name="wq_sb", tag="wq")
            wk_sb = wpool.tile([128, NC_CHUNKS, GCOLS], f16, name="wk_sb", tag="wk")
            wv_sb = wpool.tile([128, NC_CHUNKS, GCOLS], f16, name="wv_sb", tag="wv")
            wo_sb = wpool.tile([128, 4, D], f16, name="wo_sb", tag="wo")
            kt_sb = xpool.tile([128, NC_CHUNKS, S], f16, name="kt_sb", tag="ktx")
            qt_sb = xpool.tile([128, NC_CHUNKS, S], f16, name="qt_sb", tag="qtx")

            # kT/qT: [dk-on-partitions, token]; chunk j holds head 2j on
            # partitions 0:64 and head 2j+1 on 64:128
            kT_sb = projpool.tile([128, 4, S], f16, name="kT_sb", tag="kT")
            qT_sb = projpool.tile([128, 4, S], f16, name="qT_sb", tag="qT")
            # v: [token-on-partitions, head, dim(+ones col at 64)]
            v_sb = projpool.tile([128, NKB, HPC, 66], f16, name="v_sb", tag="v")
            oT_sb = projpool.tile([128, 4, S], f16, name="oT_sb", tag="oT")

            # ---- vt slice DMA + v-projection (one 128-ktok block) ----
            def vt_dma(kb):
                sl = xpool.tile([128, NC_CHUNKS, 128], f16, name="vts",
                                tag="vts", bufs=6)
                nc.sync.dma_start(out=sl[:],
                                  in_=vt_d[:, :, kb * 128:(kb + 1) * 128])
                return sl

            vt_slices = {}

            def vproj_unit(kb):
                sl = vt_slices.pop(kb)
                ps = psp.tile([128, QB], f32, name="vproj_ps", tag="pps",
                              bufs=2)
                for c in range(NC_CHUNKS):
                    nc.tensor.matmul(
                        ps[:], sl[:, c, :], wv_sb[:, c, :],
                        start=(c == 0), stop=(c == NC_CHUNKS - 1))
                nc.vector.tensor_copy(
                    v_sb[:, kb, :, 0:64],
                    ps[:].rearrange("p (h d) -> p h d", h=HPC))
                # prefetch the slice 6 ahead — emitted here so the 6-buf
                # rotation's write-after-read ordering is tracked correctly
                if kb + 6 < NKB:
                    vt_slices[kb + 6] = vt_dma(kb + 6)

            # ---- DMAs in priority order (first-needed first) ----
            nc.sync.dma_start(out=wk_sb[:], in_=wk_d[:])
            for c in range(NC_CHUNKS):
                nc.sync.dma_start(out=kt_sb[:, c, 0:1024], in_=kt_d[:, c, 0:1024])
            nc.sync.dma_start(out=wq_sb[:], in_=wq_d[:])
            for c in range(NC_CHUNKS):
                nc.sync.dma_start(out=qt_sb[:, c, 0:QB], in_=qt_d[:, c, 0:QB])
            nc.sync.dma_start(out=wv_sb[:], in_=wv_d[:])
            for kb in range(0, 4):
                vt_slices[kb] = vt_dma(kb)
            for c in range(NC_CHUNKS):
                nc.sync.dma_start(out=kt_sb[:, c, 1024:2048],
                                  in_=kt_d[:, c, 1024:2048])
            for kb in range(4, 6):
                vt_slices[kb] = vt_dma(kb)
            for c in range(NC_CHUNKS):
                nc.sync.dma_start(out=qt_sb[:, c, QB:2 * QB],
                                  in_=qt_d[:, c, QB:2 * QB])
            for n in range(2, NQB):
                for c in range(NC_CHUNKS):
                    nc.sync.dma_start(out=qt_sb[:, c, n * QB:(n + 1) * QB],
                                      in_=qt_d[:, c, n * QB:(n + 1) * QB])
            nc.sync.dma_start(out=wo_sb[:], in_=wo_d[:])

            # ---- ACT exp-table pre-warm (overlaps the DMA head) ----
            warm = workpool.tile([1, 8], f32, name="warm", tag="warm", bufs=1)
            nc.vector.memset(warm[:], 0.0)
            nc.scalar.activation(warm[:], warm[:], EXP)

            # ---- ones columns of v (only cols 64:66, not the whole tile) ----
            for kb in range(NKB):
                nc.vector.memset(v_sb[:, kb, :, 64:66], 1.0)

            # ---- kT/qT projection unit builders ----
            def proj_half(w_sb, xt_sb, dst, m, n, half, ps_holder):
                # half a kT/qT projection tile: 4 of 8 contraction chunks
                if half == 0:
                    ps_holder[0] = psp.tile([128, QB], f32, name="proj_ps",
                                            tag="pps", bufs=2)
                ps = ps_holder[0]
                for c in range(4 * half, 4 * half + 4):
                    nc.tensor.matmul(
                        ps[:],
                        w_sb[:, c, m * 128:(m + 1) * 128],
                        xt_sb[:, c, n * QB:(n + 1) * QB],
                        start=(c == 0), stop=(c == NC_CHUNKS - 1))
                if half == 1:
                    nc.vector.tensor_copy(dst[:, m, n * QB:(n + 1) * QB],
                                          ps[:])

            def proj_unit(w_sb, xt_sb, dst, m, n):
                holder = [None]
                proj_half(w_sb, xt_sb, dst, m, n, 0, holder)
                proj_half(w_sb, xt_sb, dst, m, n, 1, holder)

            def proj_halves(w_sb, xt_sb, dst, m, n):
                holder = [None]
                return [
                    lambda: proj_half(w_sb, xt_sb, dst, m, n, 0, holder),
                    lambda: proj_half(w_sb, xt_sb, dst, m, n, 1, holder),
                ]

            def y_unit(t):
                # output projection for token block t (needs oT of all pairs)
                y_sb = workpool.tile([128, D], f16, name="y_sb", tag="y",
                                     bufs=2)
                for n2 in range(2):
                    ps = psp.tile([128, QB], f32, name="y_ps", tag="pps",
                                  bufs=2)
                    for c2 in range(4):
                        nc.tensor.matmul(
                            ps[:],
                            oT_sb[:, c2, t * 128:(t + 1) * 128],
                            wo_sb[:, c2, n2 * QB:(n2 + 1) * QB],
                            start=(c2 == 0), stop=(c2 == 3))
                    nc.vector.tensor_copy(y_sb[:, n2 * QB:(n2 + 1) * QB],
                                          ps[:])
                nc.sync.dma_start(out=yp_d[t * 128:(t + 1) * 128, :],
                                  in_=y_sb[:])

            # ---- attention pair with deadline-scheduled interleave ----
            def attention_pair(j, sched):
                # sched: list of (due_it, closure), sorted by due_it; every
                # closure is emitted no later than pair-local iteration
                # due_it (its 0..63); pops always take the front so
                # multi-closure units stay contiguous. One opportunistic
                # pop per odd iteration pads the PE between attnV waits.
                h0, h1 = 2 * j, 2 * j + 1
                unnorm0 = workpool.tile([64, NQB, QB], f16, name="unnorm0",
                                        tag="unnorm0", bufs=1)
                unnorm1 = workpool.tile([64, NQB, QB], f16, name="unnorm1",
                                        tag="unnorm1", bufs=1)
                it = 0
                for qb in range(NQB):
                    out0 = psp.tile([128, QB], f32, name="out0", tag="out0",
                                    bufs=1)
                    out1 = psp.tile([128, QB], f32, name="out1", tag="out1",
                                    bufs=1)
                    for kb in range(NKB):
                        while sched and sched[0][0] <= it:
                            sched.pop(0)[1]()
                        sb2 = psp.tile([128, 2, QB], f32, name="sb2",
                                       tag="sbig", bufs=2)
                        # row-packed score pair: head h0 on PE rows 0:64,
                        # head h1 on rows 64:128 (concurrent in the array)
                        nc.tensor.matmul(
                            sb2[:, 0, :],
                            kT_sb[0:64, j, kb * 128:(kb + 1) * 128],
                            qT_sb[0:64, j, qb * QB:(qb + 1) * QB],
                            start=True, stop=True)
                        nc.tensor.matmul(
                            sb2[:, 1, :],
                            kT_sb[64:128, j, kb * 128:(kb + 1) * 128],
                            qT_sb[64:128, j, qb * QB:(qb + 1) * QB],
                            start=True, stop=True)
                        pT = workpool.tile([128, 2, QB], f16, name="pT",
                                           tag="pT", bufs=3)
                        nc.scalar.activation(
                            pT[:].rearrange("p a b -> p (a b)"),
                            sb2[:].rearrange("p a b -> p (a b)"),
                            EXP, scale=0.125)
                        if sched and it % 2 == 1:
                            sched.pop(0)[1]()
                        nc.tensor.matmul(
                            out0[0:65, :], v_sb[:, kb, h0, 0:65], pT[:, 0, :],
                            start=(kb == 0), stop=(kb == NKB - 1))
                        nc.tensor.matmul(
                            out1[0:65, :], v_sb[:, kb, h1, 0:65], pT[:, 1, :],
                            start=(kb == 0), stop=(kb == NKB - 1))
                        it += 1
                    # stage to SBUF (frees the PSUM accumulators)
                    db0 = workpool.tile([1, QB], f32, name="db0", tag="db0",
                                        bufs=1)
                    db1 = workpool.tile([1, QB], f32, name="db1", tag="db1",
                                        bufs=1)
                    nc.vector.tensor_copy(unnorm0[:, qb, :], out0[0:64, :])
                    nc.vector.tensor_copy(db0[:], out0[64:65, :])
                    nc.vector.tensor_copy(unnorm1[:, qb, :], out1[0:64, :])
                    nc.vector.tensor_copy(db1[:], out1[64:65, :])
                    # normalize immediately (cheap now: approx recip on DVE,
                    # broadcast + multiply on GpSimd)
                    rcp0 = workpool.tile([1, QB], f32, name="rcp0",
                                         tag="rcp0", bufs=1)
                    rcp1 = workpool.tile([1, QB], f32, name="rcp1",
                                         tag="rcp1", bufs=1)
                    nc.vector.reciprocal_approx_fast(out=rcp0[:], in_=db0[:])
                    nc.vector.reciprocal_approx_fast(out=rcp1[:], in_=db1[:])
                    rcph = workpool.tile([1, 2, QB], f16, name="rcph",
                                         tag="rcph", bufs=1)
                    nc.vector.tensor_copy(rcph[:, 0, :], rcp0[:])
                    nc.vector.tensor_copy(rcph[:, 1, :], rcp1[:])
                    rbc = workpool.tile([64, 2, QB], f16, name="rbc",
                                        tag="rbc", bufs=1)
                    nc.gpsimd.partition_broadcast(rbc[:, 0, :],
                                                  rcph[0:1, 0, :])
                    nc.gpsimd.partition_broadcast(rbc[:, 1, :],
                                                  rcph[0:1, 1, :])
                    nc.gpsimd.tensor_mul(
                        oT_sb[0:64, j, qb * QB:(qb + 1) * QB],
                        unnorm0[:, qb, :], rbc[:, 0, :])
                    nc.gpsimd.tensor_mul(
                        oT_sb[64:128, j, qb * QB:(qb + 1) * QB],
                        unnorm1[:, qb, :], rbc[:, 1, :])
                while sched:
                    sched.pop(0)[1]()

            # ---- PE prefix: minimum work before the first exp ----
            proj_unit(wk_sb, kt_sb, kT_sb, 0, 0)   # kT chunk 0, tokens 0:512
            proj_unit(wq_sb, qt_sb, qT_sb, 0, 0)   # qT chunk 0, tokens 0:512
            for kb in range(4):                    # v blocks 0..3, all heads
                vproj_unit(kb)

            # ---- schedules (pair-local its 0..63) ----
            sched0 = []
            for kb in range(4, NKB):
                sched0.append((kb, lambda kb=kb: vproj_unit(kb)))
            for n in range(1, NQB):  # kT(0, n) before scores reach kb=4n
                for half, fn in enumerate(proj_halves(wk_sb, kt_sb, kT_sb, 0, n)):
                    sched0.append((4 * n - 2 + half, fn))
            for n in range(1, NQB):  # qT(0, n) before qb=n starts
                for half, fn in enumerate(proj_halves(wq_sb, qt_sb, qT_sb, 0, n)):
                    sched0.append((16 * n - 3 + half, fn))
            due = 30
            for n in range(NQB):     # kT chunk 1 + qT(1, 0) during pair 0
                for fn in proj_halves(wk_sb, kt_sb, kT_sb, 1, n):
                    sched0.append((due, fn)); due += 2
            for fn in proj_halves(wq_sb, qt_sb, qT_sb, 1, 0):
                sched0.append((due, fn)); due += 2

            def mid_sched(this, nxt):
                # qT(this, n>=1) just in time; kT(next)+qT(next, 0) spread
                sched = []
                for n in range(1, NQB):
                    for half, fn in enumerate(
                            proj_halves(wq_sb, qt_sb, qT_sb, this, n)):
                        sched.append((16 * n - 3 + half, fn))
                due = 20
                for n in range(NQB):
                    for fn in proj_halves(wk_sb, kt_sb, kT_sb, nxt, n):
                        sched.append((due, fn)); due += 2
                for fn in proj_halves(wq_sb, qt_sb, qT_sb, nxt, 0):
                    sched.append((due, fn)); due += 2
                return sched

            sched1 = mid_sched(1, 2)
            sched2 = mid_sched(2, 3)

            sched3 = []
            for n in range(1, NQB):
                for half, fn in enumerate(proj_halves(wq_sb, qt_sb, qT_sb, 3, n)):
                    sched3.append((16 * n - 3 + half, fn))
            # y(t) after normalize(pair3, qb=t//4), i.e. during qb+1
            for t in range(12):
                sched3.append((16 * (t // 4 + 1) + 2 + 3 * (t % 4),
                               lambda t=t: y_unit(t)))

            for j, sched in enumerate((sched0, sched1, sched2, sched3)):
                sched.sort(key=lambda e: e[0])
                attention_pair(j, sched)

            # ---- tail: last token blocks of the output projection ----
            for t in range(12, 16):
                y_unit(t)

    nc.compile()
    _prog_cache[key] = nc
    return nc


def _chunk_pT(x):
    """[S, D] -> [128, D//128, S] fp16 (X^T chunked: out[p, c, t] = x[t, 128c+p])."""
    return np.ascontiguousarray(x.reshape(S, NC_CHUNKS, 128).transpose(2, 1, 0))


def _chunk_w(w):
    """[D, GCOLS] -> [128, 8, GCOLS]: out[p, c, m] = w[128c+p, m]."""
    return np.ascontiguousarray(
        w.reshape(NC_CHUNKS, 128, w.shape[1]).transpose(1, 0, 2))


def prepare_in_maps(Q, K, V, W_Q, W_K, W_V, W_O):
    f16 = np.float16
    qt = [_chunk_pT(Q[b].astype(f16)) for b in range(B)]
    kt = [_chunk_pT(K[b].astype(f16)) for b in range(B)]
    vt = [_chunk_pT(V[b].astype(f16)) for b in range(B)]
    wq = [_chunk_w(W_Q[:, g * GCOLS:(g + 1) * GCOLS].astype(f16)) for g in range(2)]
    wk = [_chunk_w(W_K[:, g * GCOLS:(g + 1) * GCOLS].astype(f16)) for g in range(2)]
    wv = [_chunk_w(W_V[:, g * GCOLS:(g + 1) * GCOLS].astype(f16)) for g in range(2)]
    # wo rows for group g, chunked: [128, 4, D]
    wo = [np.ascontiguousarray(
        W_O[g * GCOLS:(g + 1) * GCOLS, :].astype(f16)
        .reshape(4, 128, D).transpose(1, 0, 2)) for g in range(2)]
    in_maps = []
    for c in range(NCORES):
        b, g = c // 2, c % 2
        in_maps.append({
            "qt": qt[b], "kt": kt[b], "vt": vt[b],
            "wq": wq[g], "wk": wk[g], "wv": wv[g], "wo": wo[g],
        })
    return in_maps


def execute(nc, in_maps):
    from concourse.bass_utils import run_bass_kernel_spmd
    return run_bass_kernel_spmd(nc, in_maps, list(range(NCORES)))


def _numpy_fallback(Q, K, V, mask, W_Q, W_K, W_V, W_O):
    import math
    B_, S1, _ = Q.shape
    q = (Q.reshape(-1, D) @ W_Q).reshape(B_, S1, H, DK).transpose(0, 2, 1, 3)
    k = (K.reshape(-1, D) @ W_K).reshape(B_, S1, H, DK).transpose(0, 2, 1, 3)
    v = (V.reshape(-1, D) @ W_V).reshape(B_, S1, H, DK).transpose(0, 2, 1, 3)
    out = np.empty((B_, H, S1, DK), np.float32)
    for b in range(B_):
        for h in range(H):
            s = (q[b, h] @ k[b, h].T) / math.sqrt(DK)
            s = np.where(mask[b] == 0, np.float32(-1e9), s)
            s = s - s.max(axis=-1, keepdims=True)
            e = np.exp(s)
            p = e / e.sum(axis=-1, keepdims=True)
            out[b, h] = p @ v[b, h]
    o = out.transpose(0, 2, 1, 3).reshape(B_, S1, D)
    return (o.reshape(-1, D) @ W_O).reshape(B_, S1, D).astype(np.float32)


def kernel(Q, K, V, mask, W_Q, W_K, W_V, W_O):
    Q = np.asarray(Q); K = np.asarray(K); V = np.asarray(V)
    mask = np.asarray(mask)
    W_Q = np.asarray(W_Q); W_K = np.asarray(W_K)
    W_V = np.asarray(W_V); W_O = np.asarray(W_O)
    if (mask == 0).any():
        # spec guarantees an all-ones mask; this path is correctness insurance
        return _numpy_fallback(Q, K, V, mask, W_Q, W_K, W_V, W_O)
    nc = build_program()
    in_maps = prepare_in_maps(Q, K, V, W_Q, W_K, W_V, W_O)
    res = execute(nc, in_maps)
    out = np.empty((B, S, D), np.float32)
    for b in range(B):
        out[b] = (res.results[2 * b]["yp"].astype(np.float32)
                  + res.results[2 * b + 1]["yp"].astype(np.float32))
    return out


# revision 23
# speedup vs baseline: 1.0467x; 1.0467x over previous
"""Multi-head attention (B=4, S=2048, d_model=1024, H=16) on 8 TRN2 NeuronCores.

Sharding: tensor-parallel over heads x data-parallel over batch.
Core c handles batch b=c//2 and head group g=c%2 (8 heads = 512 of the
1024 d_model columns of W_Q/W_K/W_V, and 512 rows of W_O). Each core
produces a partial output Y_partial[b] = O_g @ W_O[g-rows, :] in fp16;
the host sums the two partials per batch in fp32.

Device-side dataflow per core (matmul operands fp16, accum fp32):
  - the attention phase is exp-bound (ScalarE ~1.1us per 128x1024 tile);
    projections and the output projection are interleaved into the PE's
    idle slots of that phase via a deadline-scheduled closure list
  - per head pair j, per 128-ktok block: scores^T = k^T.T q^T (row-tiled
    pair, concurrent in the PE array) -> exp(scale=1/8) -> P^T fp16
    -> out^T_ext += [v_h | 1].T @ P^T (row 64 = softmax denominator)
  - normalize: reciprocal_approx_fast on DVE, broadcast + multiply on
    the (otherwise idle) GpSimd engine
  - vt streams through a 6-deep rotating SBUF slice buffer (16 slices
    of 128 ktok); kt/qt/weights are SBUF-resident
"""

import numpy as np

B = 4
S = 2048
D = 1024
H = 16
DK = 64
NCORES = 8
HPC = 8          # heads per core
GCOLS = 512      # d_model columns per head group
QB = 512         # q-token block (PSUM bank free dim)
NQB = S // QB    # 4
NKB = S // 128   # 16 k-token blocks
NC_CHUNKS = D // 128  # 8 contraction chunks

_prog_cache = {}


def build_program(reps=1):
    """Build + compile the SPMD program."""
    key = (reps,)
    if key in _prog_cache:
        return _prog_cache[key]

    import concourse.bacc as bacc
    import concourse.mybir as mybir
    from concourse.tile import TileContext

    f16 = mybir.dt.float16
    f32 = mybir.dt.float32
    EXP = mybir.ActivationFunctionType.Exp

    nc = bacc.Bacc("TRN2", target_bir_lowering=False, debug=False,
                   num_devices=NCORES)

    qt_d = nc.dram_tensor("qt", [128, NC_CHUNKS, S], f16, kind="ExternalInput").ap()
    kt_d = nc.dram_tensor("kt", [128, NC_CHUNKS, S], f16, kind="ExternalInput").ap()
    vt_d = nc.dram_tensor("vt", [128, NC_CHUNKS, S], f16, kind="ExternalInput").ap()
    wq_d = nc.dram_tensor("wq", [128, NC_CHUNKS, GCOLS], f16, kind="ExternalInput").ap()
    wk_d = nc.dram_tensor("wk", [128, NC_CHUNKS, GCOLS], f16, kind="ExternalInput").ap()
    wv_d = nc.dram_tensor("wv", [128, NC_CHUNKS, GCOLS], f16, kind="ExternalInput").ap()
    wo_d = nc.dram_tensor("wo", [128, 4, D], f16, kind="ExternalInput").ap()
    yp_d = nc.dram_tensor("yp", [S, D], f16, kind="ExternalOutput").ap()

    with TileContext(nc) as tc:
        with tc.tile_pool(name="weights", bufs=1) as wpool, \
             tc.tile_pool(name="xin", bufs=1) as xpool, \
             tc.tile_pool(name="proj", bufs=1) as projpool, \
             tc.tile_pool(name="work", bufs=2) as workpool, \
             tc.tile_pool(name="psum", bufs=1, space="PSUM") as psp:

            wq_sb = wpool.tile([128, NC_CHUNKS, GCOLS], f16, name="wq_sb", tag="wq")
            wk_sb = wpool.tile([128, NC_CHUNKS, GCOLS], f16, name="wk_sb", tag="wk")
            wv_sb = wpool.tile([128, NC_CHUNKS, GCOLS], f16, name="wv_sb", tag="wv")
            wo_sb = wpool.tile([128, 4, D], f16, name="wo_sb", tag="wo")
            kt_sb = xpool.tile([128, NC_CHUNKS, S], f16, name="kt_sb", tag="ktx")
            qt_sb = xpool.tile([128, NC_CHUNKS, S], f16, name="qt_sb", tag="qtx")

            # kT/qT: [dk-on-partitions, token]; chunk j holds head 2j on
            # partitions 0:64 and head 2j+1 on 64:128
            kT_sb = projpool.tile([128, 4, S], f16, name="kT_sb", tag="kT")
            qT_sb = projpool.tile([128, 4, S], f16, name="qT_sb", tag="qT")
            # v: [token-on-partitions, head, dim(+ones col at 64)]
            v_sb = projpool.tile([128, NKB, HPC, 66], f16, name="v_sb", tag="v")
            oT_sb = projpool.tile([128, 4, S], f16, name="oT_sb", tag="oT")

            # ---- vt slice DMA + v-projection (one 128-ktok block) ----
            def vt_dma(kb):
                sl = xpool.tile([128, NC_CHUNKS, 128], f16, name="vts",
                                tag="vts", bufs=9)
                nc.sync.dma_start(out=sl[:],
                                  in_=vt_d[:, :, kb * 128:(kb + 1) * 128])
                return sl

            vt_slices = {}

            def vproj_unit(kb):
                sl = vt_slices.pop(kb)
                ps = psp.tile([128, QB], f32, name="vproj_ps", tag="pps",
                              bufs=2)
                for c in range(NC_CHUNKS):
                    nc.tensor.matmul(
                        ps[:], sl[:, c, :], wv_sb[:, c, :],
                        start=(c == 0), stop=(c == NC_CHUNKS - 1))
                nc.vector.tensor_copy(
                    v_sb[:, kb, :, 0:64],
                    ps[:].rearrange("p (h d) -> p h d", h=HPC))
                # prefetch the slice 9 ahead — emitted here so the 9-buf
                # rotation's write-after-read ordering is tracked correctly
                if kb + 9 < NKB:
                    vt_slices[kb + 9] = vt_dma(kb + 9)

            # ---- DMAs in priority order (first-needed first) ----
            # wk chunk c + kt chunk c interleaved so the first projection
            # matmul starts after ~0.5 MB instead of ~3 MB
            for c in range(NC_CHUNKS):
                nc.sync.dma_start(out=wk_sb[:, c, :], in_=wk_d[:, c, :])
                nc.sync.dma_start(out=kt_sb[:, c, 0:1024], in_=kt_d[:, c, 0:1024])
            nc.sync.dma_start(out=wq_sb[:], in_=wq_d[:])
            for c in range(NC_CHUNKS):
                nc.sync.dma_start(out=qt_sb[:, c, 0:QB], in_=qt_d[:, c, 0:QB])
            nc.sync.dma_start(out=wv_sb[:], in_=wv_d[:])
            for kb in range(0, 4):
                vt_slices[kb] = vt_dma(kb)
            for c in range(NC_CHUNKS):
                nc.sync.dma_start(out=kt_sb[:, c, 1024:2048],
                                  in_=kt_d[:, c, 1024:2048])
            for kb in range(4, 9):
                vt_slices[kb] = vt_dma(kb)
            for c in range(NC_CHUNKS):
                nc.sync.dma_start(out=qt_sb[:, c, QB:2 * QB],
                                  in_=qt_d[:, c, QB:2 * QB])
            for n in range(2, NQB):
                for c in range(NC_CHUNKS):
                    nc.sync.dma_start(out=qt_sb[:, c, n * QB:(n + 1) * QB],
                                      in_=qt_d[:, c, n * QB:(n + 1) * QB])
            nc.sync.dma_start(out=wo_sb[:], in_=wo_d[:])

            # ---- ACT exp-table pre-warm (overlaps the DMA head) ----
            warm = workpool.tile([1, 8], f32, name="warm", tag="warm", bufs=1)
            nc.vector.memset(warm[:], 0.0)
            nc.scalar.activation(warm[:], warm[:], EXP)
            # fp32 ones row for the PE-based broadcast of the final qb's
            # reciprocal (the gpsimd broadcast is DMA-backed, ~8us latency)
            ones64 = workpool.tile([1, 64], f32, name="ones64", tag="ones64",
                                   bufs=1)
            nc.vector.memset(ones64[:], 1.0)
            # ~2.5us of dummy matmuls: flips the PE HAM clock-gate to 8/8
            # while the head DMAs land, so the real prefix runs at 2.4 GHz
            wdat = workpool.tile([128, 256], f16, name="wdat", tag="wdat",
                                 bufs=1)
            nc.vector.memset(wdat[:], 0.0)
            for _ in range(10):
                wps = psp.tile([128, QB], f32, name="wps", tag="pps",
                               bufs=2)
                nc.tensor.matmul(wps[0:64, 0:256], wdat[:, 0:64],
                                 wdat[:], start=True, stop=True)

            # ---- ones columns of v ----
            for kb in range(NKB):
                nc.vector.memset(v_sb[:, kb, :, :], 1.0)

            # ---- kT/qT projection unit builders ----
            def proj_half(w_sb, xt_sb, dst, m, n, half, ps_holder):
                # half a kT/qT projection tile: 4 of 8 contraction chunks
                if half == 0:
                    ps_holder[0] = psp.tile([128, QB], f32, name="proj_ps",
                                            tag="pps", bufs=2)
                ps = ps_holder[0]
                for c in range(4 * half, 4 * half + 4):
                    nc.tensor.matmul(
                        ps[:],
                        w_sb[:, c, m * 128:(m + 1) * 128],
                        xt_sb[:, c, n * QB:(n + 1) * QB],
                        start=(c == 0), stop=(c == NC_CHUNKS - 1))
                if half == 1:
                    nc.vector.tensor_copy(dst[:, m, n * QB:(n + 1) * QB],
                                          ps[:])

            def proj_unit(w_sb, xt_sb, dst, m, n):
                holder = [None]
                proj_half(w_sb, xt_sb, dst, m, n, 0, holder)
                proj_half(w_sb, xt_sb, dst, m, n, 1, holder)

            def proj_halves(w_sb, xt_sb, dst, m, n):
                holder = [None]
                return [
                    lambda: proj_half(w_sb, xt_sb, dst, m, n, 0, holder),
                    lambda: proj_half(w_sb, xt_sb, dst, m, n, 1, holder),
                ]

            def y_unit(t):
                # output projection for token block t (needs oT of all pairs)
                y_sb = workpool.tile([128, D], f16, name="y_sb", tag="y",
                                     bufs=1)
                for n2 in range(2):
                    ps = psp.tile([128, QB], f32, name="y_ps", tag="pps",
                                  bufs=2)
                    for c2 in range(4):
                        nc.tensor.matmul(
                            ps[:],
                            oT_sb[:, c2, t * 128:(t + 1) * 128],
                            wo_sb[:, c2, n2 * QB:(n2 + 1) * QB],
                            start=(c2 == 0), stop=(c2 == 3))
                    nc.vector.tensor_copy(y_sb[:, n2 * QB:(n2 + 1) * QB],
                                          ps[:])
                nc.sync.dma_start(out=yp_d[t * 128:(t + 1) * 128, :],
                                  in_=y_sb[:])

            # ---- attention pair with deadline-scheduled interleave ----
            def attention_pair(j, sched):
                # sched: list of (due_it, closure), sorted by due_it; every
                # closure is emitted no later than pair-local iteration
                # due_it (its 0..63); pops always take the front so
                # multi-closure units stay contiguous. One opportunistic
                # pop per odd iteration pads the PE between attnV waits.
                h0, h1 = 2 * j, 2 * j + 1
                unnorm0 = workpool.tile([64, NQB, QB], f16, name="unnorm0",
                                        tag="unnorm0", bufs=1)
                unnorm1 = workpool.tile([64, NQB, QB], f16, name="unnorm1",
                                        tag="unnorm1", bufs=1)
                it = 0
                for qb in range(NQB):
                    out0 = psp.tile([128, QB], f32, name="out0", tag="out0",
                                    bufs=1)
                    out1 = psp.tile([128, QB], f32, name="out1", tag="out1",
                                    bufs=1)
                    sb2s = [None, None]   # scores run one kb ahead of attnV

                    def scores(kb, qb=qb):
                        sb2 = psp.tile([128, 2, QB], f32, name="sb2",
                                       tag="sbig", bufs=2)
                        sb2s[kb % 2] = sb2
                        # row-packed score pair: head h0 on PE rows 0:64,
                        # head h1 on rows 64:128 (concurrent in the array)
                        nc.tensor.matmul(
                            sb2[:, 0, :],
                            kT_sb[0:64, j, kb * 128:(kb + 1) * 128],
                            qT_sb[0:64, j, qb * QB:(qb + 1) * QB],
                            start=True, stop=True)
                        nc.tensor.matmul(
                            sb2[:, 1, :],
                            kT_sb[64:128, j, kb * 128:(kb + 1) * 128],
                            qT_sb[64:128, j, qb * QB:(qb + 1) * QB],
                            start=True, stop=True)

                    for kb in range(NKB):
                        while sched and sched[0][0] <= it:
                            sched.pop(0)[1]()
                        if kb == 0:
                            scores(0)
                        if kb < NKB - 1:
                            scores(kb + 1)
                        pT = workpool.tile([128, 2, QB], f16, name="pT",
                                           tag="pT", bufs=3)
                        nc.scalar.activation(
                            pT[:].rearrange("p a b -> p (a b)"),
                            sb2s[kb % 2][:].rearrange("p a b -> p (a b)"),
                            EXP, scale=0.125)
                        if sched and it % 2 == 1 and len(sched) > 8:
                            sched.pop(0)[1]()
                        nc.tensor.matmul(
                            out0[0:65, :], v_sb[:, kb, h0, 0:65], pT[:, 0, :],
                            start=(kb == 0), stop=(kb == NKB - 1))
                        nc.tensor.matmul(
                            out1[0:65, :], v_sb[:, kb, h1, 0:65], pT[:, 1, :],
                            start=(kb == 0), stop=(kb == NKB - 1))
                        it += 1
                    # stage to SBUF; out0's bank frees after its two copies
                    db = workpool.tile([65, QB], f32, name="db", tag="db",
                                       bufs=1)
                    nc.vector.tensor_copy(unnorm0[:, qb, :], out0[0:64, :])
                    nc.vector.tensor_copy(db[0:1, :], out0[64:65, :])
                    nc.vector.tensor_copy(unnorm1[:, qb, :], out1[0:64, :])
                    nc.vector.tensor_copy(db[64:65, :], out1[64:65, :])
                    # normalize immediately: approx recip on DVE, then
                    # broadcast + multiply on GpSimd (pipelined mid-kernel);
                    # the very last qb uses the PE for the broadcast and DVE
                    # for the multiplies -- the gpsimd broadcast is a
                    # DMA-backed op with ~8us latency, fatal on the tail
                    rq = workpool.tile([65, QB], f32, name="rq",
                                       tag="rq", bufs=1)
                    nc.vector.reciprocal(rq[:], db[:])
                    last = (j == 3 and qb == NQB - 1)
                    rbc = workpool.tile([64, 2, QB], f16, name="rbc",
                                        tag="rbc", bufs=1)
                    if last:
                        rcp1 = workpool.tile([1, QB], f32, name="rcp1",
                                             tag="rcp1", bufs=1)
                        nc.vector.tensor_copy(rcp1[:], rq[64:65, :])
                        for hi, rcp in enumerate((rq[0:1, :], rcp1[:])):
                            bps = psp.tile([128, QB], f32, name="bps",
                                           tag="pps", bufs=2)
                            nc.tensor.matmul(bps[0:64, :], ones64[:],
                                             rcp, start=True, stop=True)
                            nc.vector.tensor_copy(rbc[:, hi, :],
                                                  bps[0:64, :])
                        nc.vector.tensor_mul(
                            oT_sb[0:64, j, qb * QB:(qb + 1) * QB],
                            unnorm0[:, qb, :], rbc[:, 0, :])
                        nc.vector.tensor_mul(
                            oT_sb[64:128, j, qb * QB:(qb + 1) * QB],
                            unnorm1[:, qb, :], rbc[:, 1, :])
                    else:
                        rcph = workpool.tile([1, 2, QB], f16, name="rcph",
                                             tag="rcph", bufs=1)
                        nc.vector.tensor_copy(rcph[:, 0, :], rq[0:1, :])
                        nc.vector.tensor_copy(rcph[:, 1, :], rq[64:65, :])
                        nc.gpsimd.partition_broadcast(rbc[:, 0, :],
                                                      rcph[0:1, 0, :])
                        nc.gpsimd.partition_broadcast(rbc[:, 1, :],
                                                      rcph[0:1, 1, :])
                        nc.vector.tensor_mul(
                            oT_sb[0:64, j, qb * QB:(qb + 1) * QB],
                            unnorm0[:, qb, :], rbc[:, 0, :])
                        nc.vector.tensor_mul(
                            oT_sb[64:128, j, qb * QB:(qb + 1) * QB],
                            unnorm1[:, qb, :], rbc[:, 1, :])
                while sched:
                    sched.pop(0)[1]()

            # ---- PE prefix: minimum work before the first exp ----
            proj_unit(wk_sb, kt_sb, kT_sb, 0, 0)   # kT chunk 0, tokens 0:512
            proj_unit(wq_sb, qt_sb, qT_sb, 0, 0)   # qT chunk 0, tokens 0:512
            vproj_unit(0)                          # v block 0, all heads

            # ---- schedules (pair-local its 0..63) ----
            sched0 = []
            for kb in range(1, NKB):
                sched0.append((kb, lambda kb=kb: vproj_unit(kb)))
            for n in range(1, NQB):  # kT(0, n) before scores reach kb=4n
                for half, fn in enumerate(proj_halves(wk_sb, kt_sb, kT_sb, 0, n)):
                    sched0.append((4 * n - 2 + half, fn))
            for n in range(1, NQB):  # qT(0, n) before qb=n starts
                for half, fn in enumerate(proj_halves(wq_sb, qt_sb, qT_sb, 0, n)):
                    sched0.append((16 * n - 3 + half, fn))
            due = 30
            for n in range(NQB):     # kT chunk 1 + qT(1, 0) during pair 0
                for fn in proj_halves(wk_sb, kt_sb, kT_sb, 1, n):
                    sched0.append((due, fn)); due += 2
            for fn in proj_halves(wq_sb, qt_sb, qT_sb, 1, 0):
                sched0.append((due, fn)); due += 2

            def mid_sched(this, nxt):
                # qT(this, n>=1) just in time; kT(next)+qT(next, 0) spread
                sched = []
                for n in range(1, NQB):
                    for half, fn in enumerate(
                            proj_halves(wq_sb, qt_sb, qT_sb, this, n)):
                        sched.append((16 * n - 3 + half, fn))
                due = 18
                for n in range(NQB):
                    for fn in proj_halves(wk_sb, kt_sb, kT_sb, nxt, n):
                        sched.append((due, fn)); due += 3
                for fn in proj_halves(wq_sb, qt_sb, qT_sb, nxt, 0):
                    sched.append((due, fn)); due += 3
                return sched

            sched1 = mid_sched(1, 2)
            sched2 = mid_sched(2, 3)

            sched3 = []
            for n in range(1, NQB):
                for half, fn in enumerate(proj_halves(wq_sb, qt_sb, qT_sb, 3, n)):
                    sched3.append((16 * n - 3 + half, fn))
            # y(t) after normalize(pair3, qb=t//4), i.e. during qb+1
            for t in range(12):
                sched3.append((16 * (t // 4 + 1) + 2 + 3 * (t % 4),
                               lambda t=t: y_unit(t)))

            for j, sched in enumerate((sched0, sched1, sched2, sched3)):
                sched.sort(key=lambda e: e[0])
                attention_pair(j, sched)

            # ---- tail: last token blocks of the output projection ----
            for t in range(12, 16):
                y_unit(t)

    nc.compile()
    _prog_cache[key] = nc
    return nc


def _chunk_pT(x):
    """[S, D] -> [128, D//128, S] fp16 (X^T chunked: out[p, c, t] = x[t, 128c+p])."""
    return np.ascontiguousarray(x.reshape(S, NC_CHUNKS, 128).transpose(2, 1, 0))


def _chunk_w(w):
    """[D, GCOLS] -> [128, 8, GCOLS]: out[p, c, m] = w[128c+p, m]."""
    return np.ascontiguousarray(
        w.reshape(NC_CHUNKS, 128, w.shape[1]).transpose(1, 0, 2))


def prepare_in_maps(Q, K, V, W_Q, W_K, W_V, W_O):
    f16 = np.float16
    qt = [_chunk_pT(Q[b].astype(f16)) for b in range(B)]
    kt = [_chunk_pT(K[b].astype(f16)) for b in range(B)]
    vt = [_chunk_pT(V[b].astype(f16)) for b in range(B)]
    wq = [_chunk_w(W_Q[:, g * GCOLS:(g + 1) * GCOLS].astype(f16)) for g in range(2)]
    wk = [_chunk_w(W_K[:, g * GCOLS:(g + 1) * GCOLS].astype(f16)) for g in range(2)]
    wv = [_chunk_w(W_V[:, g * GCOLS:(g + 1) * GCOLS].astype(f16)) for g in range(2)]
    # wo rows for group g, chunked: [128, 4, D]
    wo = [np.ascontiguousarray(
        W_O[g * GCOLS:(g + 1) * GCOLS, :].astype(f16)
        .reshape(4, 128, D).transpose(1, 0, 2)) for g in range(2)]
    in_maps = []
    for c in range(NCORES):
        b, g = c // 2, c % 2
        in_maps.append({
            "qt": qt[b], "kt": kt[b], "vt": vt[b],
            "wq": wq[g], "wk": wk[g], "wv": wv[g], "wo": wo[g],
        })
    return in_maps


def execute(nc, in_maps):
    from concourse.bass_utils import run_bass_kernel_spmd
    return run_bass_kernel_spmd(nc, in_maps, list(range(NCORES)))


def _numpy_fallback(Q, K, V, mask, W_Q, W_K, W_V, W_O):
    import math
    B_, S1, _ = Q.shape
    q = (Q.reshape(-1, D) @ W_Q).reshape(B_, S1, H, DK).transpose(0, 2, 1, 3)
    k = (K.reshape(-1, D) @ W_K).reshape(B_, S1, H, DK).transpose(0, 2, 1, 3)
    v = (V.reshape(-1, D) @ W_V).reshape(B_, S1, H, DK).transpose(0, 2, 1, 3)
    out = np.empty((B_, H, S1, DK), np.float32)
    for b in range(B_):
        for h in range(H):
            s = (q[b, h] @ k[b, h].T) / math.sqrt(DK)
            s = np.where(mask[b] == 0, np.float32(-1e9), s)
            s = s - s.max(axis=-1, keepdims=True)
            e = np.exp(s)
            p = e / e.sum(axis=-1, keepdims=True)
            out[b, h] = p @ v[b, h]
    o = out.transpose(0, 2, 1, 3).reshape(B_, S1, D)
    return (o.reshape(-1, D) @ W_O).reshape(B_, S1, D).astype(np.float32)


def kernel(Q, K, V, mask, W_Q, W_K, W_V, W_O):
    Q = np.asarray(Q); K = np.asarray(K); V = np.asarray(V)
    mask = np.asarray(mask)
    W_Q = np.asarray(W_Q); W_K = np.asarray(W_K)
    W_V = np.asarray(W_V); W_O = np.asarray(W_O)
    if (mask == 0).any():
        # spec guarantees an all-ones mask; this path is correctness insurance
        return _numpy_fallback(Q, K, V, mask, W_Q, W_K, W_V, W_O)
    nc = build_program()
    in_maps = prepare_in_maps(Q, K, V, W_Q, W_K, W_V, W_O)

    def run_once():
        res = execute(nc, in_maps)
        out = np.empty((B, S, D), np.float32)
        for b in range(B):
            out[b] = (res.results[2 * b]["yp"].astype(np.float32)
                      + res.results[2 * b + 1]["yp"].astype(np.float32))
        return out

    # The runtime occasionally corrupts a (usually first) execution --
    # observed on the unmodified baseline too. Device runs are
    # deterministic, so run twice and accept on agreement; arbitrate
    # with extra runs otherwise.
    a = run_once()
    b_ = run_once()
    if np.array_equal(a, b_, equal_nan=True) and np.isfinite(a).all():
        return a
    for _ in range(3):
        c = run_once()
        if np.isfinite(c).all():
            if np.array_equal(c, a, equal_nan=True) or np.array_equal(
                    c, b_, equal_nan=True):
                return c
            a, b_ = b_, c
    return c if np.isfinite(c).all() else (a if np.isfinite(a).all() else b_)
